# revision 5
# baseline (speedup 1.0000x reference)
"""CMSA (cross-modal self-attention) Trainium2 Bass kernel.

Problem: two feature maps x,y of [B=4, C=256, H=64, W=64]. Per sample:
  q_y,k_y = 1x1conv(y) -> [32, N]; v_x = 1x1conv(x) -> [256, N]  (N=4096)
  att_y = softmax(q_y^T k_y); enhanced_x = v_x @ att_y^T + x
  (and symmetrically x->y). Output: (enhanced_x, enhanced_y).

Sharding: 8 independent attention problems = (4 samples) x (2 directions),
one per NeuronCore, SPMD. Per-core kernel computes one full attention.

Kernel math (per core):
  L^T[j,i] = sum_d k[d,j] q[d,i]        (k-tile stationary, q moving)
  U^T[j,i] = exp(L^T[j,i])              (unnormalized, no max-sub: |logit|<~45
                                         so exp stays well inside fp32 range)
  numer[c,i] = sum_j V[c,j] U^T[j,i]    (V^T-block stationary, U^T moving)
  denom[i]   = sum_j U^T[j,i]           (ones-column stationary)
  out[c,i]   = numer[c,i] / denom[i] + feat_v[c,i]

Matmuls run in float32r (1s+8e+11m, full PE rate: 1 cycle/row vs 4 for
fp32). Operands must be rounded to fp32r by the producing instruction, so
every matmul input tile is written by a DVE/ACT op with an fp32r output
dtype. The residual add uses the unrounded fp32 features.
"""

import numpy as np

import concourse.bass as bass
import concourse.tile as tile
from concourse import bacc, mybir
from concourse.bass_utils import run_bass_kernel_spmd
from concourse.masks import make_identity

C = 256
RD = 32
B = 4
N = 64 * 64  # 4096
NCORES = 8

IBLK = 512           # i-block (query block) size
NIB = N // IBLK      # 8
JT = 128             # j tile size
NJT = N // JT        # 32

F32 = mybir.dt.float32
F32R = mybir.dt.float32r


def _build_bass():
    nc = bacc.Bacc(
        "TRN2",
        target_bir_lowering=False,
        debug=False,
        num_devices=NCORES,
    )

    feat_qk = nc.dram_tensor("feat_qk", [C, N], F32, kind="ExternalInput").ap()
    feat_v = nc.dram_tensor("feat_v", [C, N], F32, kind="ExternalInput").ap()
    wq = nc.dram_tensor("wq", [RD, C], F32, kind="ExternalInput").ap()
    wk = nc.dram_tensor("wk", [RD, C], F32, kind="ExternalInput").ap()
    wv = nc.dram_tensor("wv", [C, C], F32, kind="ExternalInput").ap()
    bq = nc.dram_tensor("bq", [RD], F32, kind="ExternalInput").ap()
    bk = nc.dram_tensor("bk", [RD], F32, kind="ExternalInput").ap()
    bv = nc.dram_tensor("bv", [C], F32, kind="ExternalInput").ap()
    out = nc.dram_tensor("out", [C, N], F32, kind="ExternalOutput").ap()

    with tile.TileContext(nc) as tc:
        _kernel_body(nc, tc, feat_qk, feat_v, wq, wk, wv, bq, bk, bv, out)
    nc.compile()
    return nc


def _kernel_body(nc, tc, feat_qk, feat_v, wq, wk, wv, bq, bk, bv, out):
    with (
        tc.tile_pool(name="singles", bufs=1) as singles,
        tc.tile_pool(name="work", bufs=3) as work,
        tc.tile_pool(name="opool", bufs=3) as opool,
        tc.tile_pool(name="qk_psum", bufs=3, space="PSUM") as qk_psum,
        tc.tile_pool(name="av_psum", bufs=2, space="PSUM") as av_psum,
        tc.tile_pool(name="den_psum", bufs=2, space="PSUM") as den_psum,
    ):
        # ---- constants ----
        ones_col = singles.tile([128, 1], F32R, tag="ones_col")

        # [cin_inner=128, cin_outer=2, n] — needed for V proj and residual
        fv_sb = singles.tile([128, 2, N], F32, tag="fv")
        for co in range(2):
            nc.sync.dma_start(
                out=fv_sb[:, co, :], in_=feat_v[co * 128 : (co + 1) * 128, :]
            )

        # projection outputs (live for the whole kernel), fp32r for matmul
        q_sb = singles.tile([RD, N], F32R, tag="q")
        k_sb = singles.tile([RD, N], F32R, tag="k")
        vT_sb = singles.tile([128, NJT, C], F32R, tag="vT")

        # ---- projection phase (scratch freed before the main loop) ----
        with (
            tc.tile_pool(name="proj", bufs=1) as proj,
            tc.tile_pool(name="vstage", bufs=4) as vstage,
        ):
            identity = proj.tile([128, 128], F32, tag="identity")
            make_identity(nc, identity)

            ones_f32 = proj.tile([128, 1], F32, tag="ones_f32")
            nc.vector.memset(ones_f32, 1.0)
            nc.vector.tensor_copy(out=ones_col, in_=ones_f32)

            fqk_sb = proj.tile([128, 2, N], F32, tag="fqk")
            fqkr = proj.tile([128, 2, N], F32R, tag="fqkr")
            for co in range(2):
                nc.sync.dma_start(
                    out=fqk_sb[:, co, :],
                    in_=feat_qk[co * 128 : (co + 1) * 128, :],
                )
                nc.vector.tensor_copy(out=fqkr[:, co, :], in_=fqk_sb[:, co, :])

            # ---- load weights / biases ----
            wq_sb = proj.tile([RD, C], F32, tag="wq")
            wk_sb = proj.tile([RD, C], F32, tag="wk")
            wv_sb = proj.tile([128, 2, C], F32, tag="wv")  # wv rows chunked
            nc.gpsimd.dma_start(out=wq_sb, in_=wq)
            nc.gpsimd.dma_start(out=wk_sb, in_=wk)
            for co in range(2):
                nc.gpsimd.dma_start(
                    out=wv_sb[:, co, :], in_=wv[co * 128 : (co + 1) * 128, :]
                )
            bq_sb = proj.tile([RD, 1], F32, tag="bq")
            bk_sb = proj.tile([RD, 1], F32, tag="bk")
            bv_sb = proj.tile([1, C], F32, tag="bv")
            nc.gpsimd.dma_start(out=bq_sb, in_=bq.rearrange("(r o) -> r o", o=1))
            nc.gpsimd.dma_start(out=bk_sb, in_=bk.rearrange("(r o) -> r o", o=1))
            nc.gpsimd.dma_start(out=bv_sb, in_=bv.rearrange("(o c) -> o c", o=1))

            # bv broadcast to all partitions: [128, C]
            bvb_sb = proj.tile([128, C], F32, tag="bvb")
            nc.gpsimd.partition_broadcast(bvb_sb, bv_sb)

            # ---- transpose weights (PE transpose, wX^T needed as lhsT) ----
            # wqT/wkT: [cin_inner=128, cin_outer=2, RD]
            wqT_sb = proj.tile([128, 2, RD], F32R, tag="wqT")
            wkT_sb = proj.tile([128, 2, RD], F32R, tag="wkT")
            for (w_sb, wT_sb) in ((wq_sb, wqT_sb), (wk_sb, wkT_sb)):
                for co in range(2):
                    tp = qk_psum.tile([128, RD], F32, tag="qk")
                    nc.tensor.transpose(
                        tp, w_sb[:, co * 128 : (co + 1) * 128], identity[:RD, :RD]
                    )
                    nc.vector.tensor_copy(out=wT_sb[:, co, :], in_=tp)
            # wvT: [cin_inner=128, cin_outer=2, C] ; wvT[ci, co, c] = wv[c, ci]
            wvT_sb = proj.tile([128, 2, C], F32R, tag="wvT")
            for o in range(2):  # wv row chunk (c dim)
                for i in range(2):  # wv col chunk (cin dim)
                    tp = qk_psum.tile([128, 128], F32, tag="qk")
                    nc.tensor.transpose(
                        tp, wv_sb[:, o, i * 128 : (i + 1) * 128], identity
                    )
                    nc.vector.tensor_copy(
                        out=wvT_sb[:, i, o * 128 : (o + 1) * 128], in_=tp
                    )

            # ---- projections ----
            # q/k: [RD, N] = wq^T.T @ feat_qk (+ bias)
            for (wT_sb, b_sb, dst) in (
                (wqT_sb, bq_sb, q_sb),
                (wkT_sb, bk_sb, k_sb),
            ):
                for nb in range(NIB):
                    ns = bass.ts(nb, IBLK)
                    pp = qk_psum.tile([RD, IBLK], F32, tag="qk")
                    for co in range(2):
                        nc.tensor.matmul(
                            pp,
                            wT_sb[:, co, :],
                            fqkr[:, co, ns],
                            start=(co == 0),
                            stop=(co == 1),
                        )
                    nc.vector.tensor_scalar_add(
                        out=dst[:, ns], in0=pp, scalar1=b_sb
                    )

            # V^T: [j, c] tiles; vT[j, c] = sum_ci feat_v[ci, j] wv[c, ci] + bv[c]
            for jt in range(NJT):
                vp = qk_psum.tile([128, C], F32, tag="qk")
                for co in range(2):
                    fvr = vstage.tile([128, JT], F32R, tag="fvr")
                    nc.vector.tensor_copy(
                        out=fvr, in_=fv_sb[:, co, bass.ts(jt, JT)]
                    )
                    nc.tensor.matmul(
                        vp,
                        fvr,
                        wvT_sb[:, co, :],
                        start=(co == 0),
                        stop=(co == 1),
                    )
                nc.vector.tensor_add(out=vT_sb[:, jt, :], in0=vp, in1=bvb_sb)

        # ---- main attention loop over query blocks ----
        with tc.tile_pool(name="upool", bufs=36) as upool:
            _attention_loop(
                nc, upool, work, opool, qk_psum, av_psum, den_psum,
                q_sb, k_sb, vT_sb, fv_sb, ones_col, out,
            )


def _attention_loop(nc, upool, work, opool, qk_psum, av_psum, den_psum,
                    q_sb, k_sb, vT_sb, fv_sb, ones_col, out):
    Exp = mybir.ActivationFunctionType.Exp
    for nb in range(NIB):
        ns = bass.ts(nb, IBLK)

        # QK^T (transposed logits) + exp, per j tile
        u_tiles = []
        for jt in range(NJT):
            lp = qk_psum.tile([JT, IBLK], F32, tag="qk")
            nc.tensor.matmul(
                lp,
                k_sb[:, bass.ts(jt, JT)],
                q_sb[:, ns],
                start=True,
                stop=True,
            )
            ut = upool.tile([JT, IBLK], F32R, tag="u")
            nc.scalar.activation(out=ut, in_=lp, func=Exp)
            u_tiles.append(ut)

        # AV + denominator, accumulating over j tiles
        av0 = av_psum.tile([128, IBLK], F32, tag="av")
        av1 = av_psum.tile([128, IBLK], F32, tag="av")
        den = den_psum.tile([1, IBLK], F32, tag="den")
        for jt in range(NJT):
            st = jt == 0
            sp = jt == NJT - 1
            nc.tensor.matmul(
                av0, vT_sb[:, jt, 0:128], u_tiles[jt], start=st, stop=sp,
            )
            nc.tensor.matmul(
                av1, vT_sb[:, jt, 128:256], u_tiles[jt], start=st, stop=sp,
            )
            nc.tensor.matmul(
                den, ones_col, u_tiles[jt], start=st, stop=sp,
            )

        # normalize + residual + store
        recip = work.tile([1, IBLK], F32, tag="recip")
        nc.vector.reciprocal(recip, den)
        rb_sb = work.tile([128, IBLK], F32, tag="rb")
        nc.gpsimd.partition_broadcast(rb_sb, recip)
        for cc, avp in ((0, av0), (1, av1)):
            ot = opool.tile([128, IBLK], F32, tag="o")
            nc.vector.tensor_mul(ot, avp, rb_sb)
            nc.vector.tensor_add(ot, ot, fv_sb[:, cc, ns])
            nc.sync.dma_start(
                out=out[cc * 128 : (cc + 1) * 128, ns], in_=ot
            )


_NC_CACHE = None


def _get_nc():
    global _NC_CACHE
    if _NC_CACHE is None:
        _NC_CACHE = _build_bass()
    return _NC_CACHE


def kernel(x_features, y_features, wqx, bqx, wkx, bkx, wvx, bvx,
           wqy, bqy, wky, bky, wvy, bvy):
    nc = _get_nc()

    def c(a):
        return np.ascontiguousarray(np.asarray(a), dtype=np.float32)

    in_maps = []
    for b in range(B):
        xf = c(x_features[b]).reshape(C, N)
        yf = c(y_features[b]).reshape(C, N)
        # core 2b: enhanced_x[b] — attention from y features, values from x
        in_maps.append({
            "feat_qk": yf, "feat_v": xf,
            "wq": c(wqy), "wk": c(wky), "wv": c(wvx),
            "bq": c(bqy), "bk": c(bky), "bv": c(bvx),
        })
        # core 2b+1: enhanced_y[b] — attention from x features, values from y
        in_maps.append({
            "feat_qk": xf, "feat_v": yf,
            "wq": c(wqx), "wk": c(wkx), "wv": c(wvy),
            "bq": c(bqx), "bk": c(bkx), "bv": c(bvy),
        })

    res = run_bass_kernel_spmd(nc, in_maps, core_ids=list(range(NCORES)))
    outs = [r["out"].reshape(C, 64, 64) for r in res.results]
    enhanced_x = np.stack(outs[0::2], axis=0)
    enhanced_y = np.stack(outs[1::2], axis=0)
    return enhanced_x, enhanced_y


# revision 9
# speedup vs baseline: 1.3241x; 1.3241x over previous
"""CMSA (cross-modal self-attention) Trainium2 Bass kernel.

Problem: two feature maps x,y of [B=4, C=256, H=64, W=64]. Per sample:
  q_y,k_y = 1x1conv(y) -> [32, N]; v_x = 1x1conv(x) -> [256, N]  (N=4096)
  att_y = softmax(q_y^T k_y); enhanced_x = v_x @ att_y^T + x
  (and symmetrically x->y). Output: (enhanced_x, enhanced_y).

Sharding: 8 independent attention problems = (4 samples) x (2 directions),
one per NeuronCore, SPMD. Per-core kernel computes one full attention.

Kernel math (per core):
  L^T[j,i] = sum_d k[d,j] q[d,i]        (k-tile stationary, q moving)
  U^T[j,i] = exp(L^T[j,i])              (unnormalized, no max-sub: |logit|<~45
                                         so exp stays well inside fp32 range)
  numer[c,i] = sum_j V[c,j] U^T[j,i]    (V^T-block stationary, U^T moving)
  denom[i]   = sum_j U^T[j,i]           (ones-column stationary)
  out[c,i]   = numer[c,i] / denom[i] + feat_v[c,i]

Matmuls run in float32r (1s+8e+11m, full PE rate: 1 cycle/row vs 4 for
fp32). Operands must be rounded to fp32r by the producing instruction, so
every matmul input tile is written by a DVE/ACT op with an fp32r output
dtype. The residual add uses the unrounded fp32 features.
"""

import numpy as np

import concourse.bass as bass
import concourse.tile as tile
from concourse import bacc, mybir
from concourse.bass_utils import run_bass_kernel_spmd
from concourse.masks import make_identity

C = 256
RD = 32
B = 4
N = 64 * 64  # 4096
NCORES = 8

IBLK = 512           # projection n-chunk size
NIB = N // IBLK      # 8
IB2 = 1024           # i-pair block size in the attention loop
NIB2 = N // IB2      # 4
JT = 128             # j tile size
NJT = N // JT        # 32
LAG = 2              # software-pipeline lag between QK/exp and AV stages

F32 = mybir.dt.float32
F32R = mybir.dt.float32r


def _build_bass():
    nc = bacc.Bacc(
        "TRN2",
        target_bir_lowering=False,
        debug=False,
        num_devices=NCORES,
    )

    feat_qk = nc.dram_tensor("feat_qk", [C, N], F32, kind="ExternalInput").ap()
    feat_v = nc.dram_tensor("feat_v", [C, N], F32, kind="ExternalInput").ap()
    wq = nc.dram_tensor("wq", [RD, C], F32, kind="ExternalInput").ap()
    wk = nc.dram_tensor("wk", [RD, C], F32, kind="ExternalInput").ap()
    wv = nc.dram_tensor("wv", [C, C], F32, kind="ExternalInput").ap()
    bq = nc.dram_tensor("bq", [RD], F32, kind="ExternalInput").ap()
    bk = nc.dram_tensor("bk", [RD], F32, kind="ExternalInput").ap()
    bv = nc.dram_tensor("bv", [C], F32, kind="ExternalInput").ap()
    out = nc.dram_tensor("out", [C, N], F32, kind="ExternalOutput").ap()

    with tile.TileContext(nc) as tc:
        _kernel_body(nc, tc, feat_qk, feat_v, wq, wk, wv, bq, bk, bv, out)
    nc.compile()
    return nc


def _kernel_body(nc, tc, feat_qk, feat_v, wq, wk, wv, bq, bk, bv, out):
    with (
        tc.tile_pool(name="singles", bufs=1) as singles,
        tc.tile_pool(name="work", bufs=3) as work,
        tc.tile_pool(name="opool", bufs=3) as opool,
        tc.tile_pool(name="qk_psum", bufs=1, space="PSUM") as qk_psum,
        tc.tile_pool(name="av_psum", bufs=2, space="PSUM") as av_psum,
        tc.tile_pool(name="den_psum", bufs=1, space="PSUM") as den_psum,
    ):
        # ---- constants ----
        ones_col = singles.tile([128, 1], F32R, tag="ones_col")

        # [cin_inner=128, cin_outer=2, n] — needed for V proj and residual
        fv_sb = singles.tile([128, 2, N], F32, tag="fv")
        for co in range(2):
            nc.sync.dma_start(
                out=fv_sb[:, co, :], in_=feat_v[co * 128 : (co + 1) * 128, :]
            )

        # projection outputs (live for the whole kernel), fp32r for matmul
        q_sb = singles.tile([RD, N], F32R, tag="q")
        k_sb = singles.tile([RD, N], F32R, tag="k")
        vT_sb = singles.tile([128, NJT, C], F32R, tag="vT")

        # ---- projection phase (scratch freed before the main loop) ----
        with (
            tc.tile_pool(name="proj", bufs=1) as proj,
            tc.tile_pool(name="vstage", bufs=4) as vstage,
        ):
            identity = proj.tile([128, 128], F32, tag="identity")
            make_identity(nc, identity)

            ones_f32 = proj.tile([128, 1], F32, tag="ones_f32")
            nc.vector.memset(ones_f32, 1.0)
            nc.vector.tensor_copy(out=ones_col, in_=ones_f32)

            fqk_sb = proj.tile([128, 2, N], F32, tag="fqk")
            fqkr = proj.tile([128, 2, N], F32R, tag="fqkr")
            for co in range(2):
                nc.sync.dma_start(
                    out=fqk_sb[:, co, :],
                    in_=feat_qk[co * 128 : (co + 1) * 128, :],
                )
                nc.vector.tensor_copy(out=fqkr[:, co, :], in_=fqk_sb[:, co, :])

            # ---- load weights / biases ----
            wq_sb = proj.tile([RD, C], F32, tag="wq")
            wk_sb = proj.tile([RD, C], F32, tag="wk")
            wv_sb = proj.tile([128, 2, C], F32, tag="wv")  # wv rows chunked
            nc.gpsimd.dma_start(out=wq_sb, in_=wq)
            nc.gpsimd.dma_start(out=wk_sb, in_=wk)
            for co in range(2):
                nc.gpsimd.dma_start(
                    out=wv_sb[:, co, :], in_=wv[co * 128 : (co + 1) * 128, :]
                )
            bq_sb = proj.tile([RD, 1], F32, tag="bq")
            bk_sb = proj.tile([RD, 1], F32, tag="bk")
            bv_sb = proj.tile([1, C], F32, tag="bv")
            nc.gpsimd.dma_start(out=bq_sb, in_=bq.rearrange("(r o) -> r o", o=1))
            nc.gpsimd.dma_start(out=bk_sb, in_=bk.rearrange("(r o) -> r o", o=1))
            nc.gpsimd.dma_start(out=bv_sb, in_=bv.rearrange("(o c) -> o c", o=1))

            # bv broadcast to all partitions: [128, C]
            bvb_sb = proj.tile([128, C], F32, tag="bvb")
            nc.gpsimd.partition_broadcast(bvb_sb, bv_sb)

            # ---- transpose weights (PE transpose, wX^T needed as lhsT) ----
            # wqT/wkT: [cin_inner=128, cin_outer=2, RD]
            wqT_sb = proj.tile([128, 2, RD], F32R, tag="wqT")
            wkT_sb = proj.tile([128, 2, RD], F32R, tag="wkT")
            for (w_sb, wT_sb) in ((wq_sb, wqT_sb), (wk_sb, wkT_sb)):
                for co in range(2):
                    tp = qk_psum.tile([128, RD], F32, tag="qk")
                    nc.tensor.transpose(
                        tp, w_sb[:, co * 128 : (co + 1) * 128], identity[:RD, :RD]
                    )
                    nc.vector.tensor_copy(out=wT_sb[:, co, :], in_=tp)
            # wvT: [cin_inner=128, cin_outer=2, C] ; wvT[ci, co, c] = wv[c, ci]
            wvT_sb = proj.tile([128, 2, C], F32R, tag="wvT")
            for o in range(2):  # wv row chunk (c dim)
                for i in range(2):  # wv col chunk (cin dim)
                    tp = qk_psum.tile([128, 128], F32, tag="qk")
                    nc.tensor.transpose(
                        tp, wv_sb[:, o, i * 128 : (i + 1) * 128], identity
                    )
                    nc.vector.tensor_copy(
                        out=wvT_sb[:, i, o * 128 : (o + 1) * 128], in_=tp
                    )

            # ---- projections ----
            # q/k: [RD, N] = wq^T.T @ feat_qk (+ bias)
            for (wT_sb, b_sb, dst) in (
                (wqT_sb, bq_sb, q_sb),
                (wkT_sb, bk_sb, k_sb),
            ):
                for nb in range(NIB):
                    ns = bass.ts(nb, IBLK)
                    pp = qk_psum.tile([RD, IBLK], F32, tag="qk")
                    for co in range(2):
                        nc.tensor.matmul(
                            pp,
                            wT_sb[:, co, :],
                            fqkr[:, co, ns],
                            start=(co == 0),
                            stop=(co == 1),
                        )
                    nc.vector.tensor_scalar_add(
                        out=dst[:, ns], in0=pp, scalar1=b_sb
                    )

            # V^T: [j, c] tiles; vT[j, c] = sum_ci feat_v[ci, j] wv[c, ci] + bv[c]
            for jt in range(NJT):
                vp = qk_psum.tile([128, C], F32, tag="qk")
                for co in range(2):
                    fvr = vstage.tile([128, JT], F32R, tag="fvr")
                    nc.vector.tensor_copy(
                        out=fvr, in_=fv_sb[:, co, bass.ts(jt, JT)]
                    )
                    nc.tensor.matmul(
                        vp,
                        fvr,
                        wvT_sb[:, co, :],
                        start=(co == 0),
                        stop=(co == 1),
                    )
                nc.vector.tensor_add(out=vT_sb[:, jt, :], in0=vp, in1=bvb_sb)

        # ---- main attention loop over query blocks ----
        with tc.tile_pool(name="upool", bufs=8) as upool:
            _attention_loop(
                nc, upool, work, opool, qk_psum, av_psum, den_psum,
                q_sb, k_sb, vT_sb, fv_sb, ones_col, out,
            )


def _attention_loop(nc, upool, work, opool, qk_psum, av_psum, den_psum,
                    q_sb, k_sb, vT_sb, fv_sb, ones_col, out):
    Exp = mybir.ActivationFunctionType.Exp
    for nb in range(NIB2):
        ns = bass.ts(nb, IB2)

        av0 = av_psum.tile([128, IB2], F32, tag="av")
        av1 = av_psum.tile([128, IB2], F32, tag="av")
        den = den_psum.tile([1, IB2], F32, tag="den")
        u_tiles = []

        def av_step(jt):
            # AV + denominator matmuls for j tile jt, accumulating in psum
            st = jt == 0
            sp = jt == NJT - 1
            ut = u_tiles[jt]
            for h in range(2):
                hs = bass.ts(h, IB2 // 2)
                uh = ut[:, hs]
                nc.tensor.matmul(
                    av0[:, hs], vT_sb[:, jt, 0:128], uh, start=st, stop=sp,
                )
                nc.tensor.matmul(
                    av1[:, hs], vT_sb[:, jt, 128:256], uh, start=st, stop=sp,
                )
                nc.tensor.matmul(
                    den[:, hs], ones_col, uh, start=st, stop=sp,
                )

        # software pipeline: QK(jt)+exp(jt) interleaved with AV(jt-LAG)
        # keeps PE dense (no HAM cool-down) and ACT hidden behind PE
        for jt in range(NJT):
            lp = qk_psum.tile([JT, IB2], F32, tag="qk")
            for h in range(2):
                hs = bass.ts(h, IB2 // 2)
                nc.tensor.matmul(
                    lp[:, hs],
                    k_sb[:, bass.ts(jt, JT)],
                    q_sb[:, bass.ds(nb * IB2 + h * (IB2 // 2), IB2 // 2)],
                    start=True,
                    stop=True,
                )
            ut = upool.tile([JT, IB2], F32R, tag="u")
            nc.scalar.activation(out=ut, in_=lp, func=Exp)
            u_tiles.append(ut)
            if jt >= LAG:
                av_step(jt - LAG)
        for jt in range(NJT - LAG, NJT):
            av_step(jt)

        # normalize + residual + store
        recip = work.tile([1, IB2], F32, tag="recip")
        nc.vector.reciprocal(recip, den)
        rb_sb = work.tile([128, IB2], F32, tag="rb")
        nc.gpsimd.partition_broadcast(rb_sb, recip)
        for cc, avp in ((0, av0), (1, av1)):
            ot = opool.tile([128, IB2], F32, tag="o")
            nc.vector.tensor_mul(ot, avp, rb_sb)
            nc.vector.tensor_add(ot, ot, fv_sb[:, cc, ns])
            nc.sync.dma_start(
                out=out[cc * 128 : (cc + 1) * 128, ns], in_=ot
            )


_NC_CACHE = None


def _get_nc():
    global _NC_CACHE
    if _NC_CACHE is None:
        _NC_CACHE = _build_bass()
    return _NC_CACHE


def kernel(x_features, y_features, wqx, bqx, wkx, bkx, wvx, bvx,
           wqy, bqy, wky, bky, wvy, bvy):
    nc = _get_nc()

    def c(a):
        return np.ascontiguousarray(np.asarray(a), dtype=np.float32)

    in_maps = []
    for b in range(B):
        xf = c(x_features[b]).reshape(C, N)
        yf = c(y_features[b]).reshape(C, N)
        # core 2b: enhanced_x[b] — attention from y features, values from x
        in_maps.append({
            "feat_qk": yf, "feat_v": xf,
            "wq": c(wqy), "wk": c(wky), "wv": c(wvx),
            "bq": c(bqy), "bk": c(bky), "bv": c(bvx),
        })
        # core 2b+1: enhanced_y[b] — attention from x features, values from y
        in_maps.append({
            "feat_qk": xf, "feat_v": yf,
            "wq": c(wqx), "wk": c(wkx), "wv": c(wvy),
            "bq": c(bqx), "bk": c(bkx), "bv": c(bvy),
        })

    res = run_bass_kernel_spmd(nc, in_maps, core_ids=list(range(NCORES)))
    outs = [r["out"].reshape(C, 64, 64) for r in res.results]
    enhanced_x = np.stack(outs[0::2], axis=0)
    enhanced_y = np.stack(outs[1::2], axis=0)
    return enhanced_x, enhanced_y
